# revision 1
# baseline (speedup 1.0000x reference)
"""Multi-head causal attention (B=2, S=2048, D=1024, H=16, Dh=64) on 8 TRN2 cores.

Sharding: tensor-parallel over 4 head-groups x data-parallel over 2 batches.
Core c handles batch c//4, heads [4*(c%4), 4*(c%4)+4). Each core computes its
partial output projection; the host sums the 4 partials per batch (the
"all-reduce") and adds b_O.

Per-core device program (Tile framework, fp32r matmuls, fp32 accumulation):
  QT/KT = (Wq|Wk/8)^T x^T   [dhead-pair=128, seq]   (scores scale folded into K)
  V     = x Wv              [seq, 4 heads x (64 | ones-col)]
  per (q-chunk 512, head):  S^T[kt] = KT_kt^T QT_chunk  (k on partitions)
      PT = exp(S^T) (ACT, kt-paired 1024-wide calls, causal-cropped,
                     triangular/extended mask on diagonal tiles)
      Z' = [V|1]^T PT accumulated over kt   -> rows 0:64 = Z^T, row 64 = denom
      ZT = Z' * (1/denom)   (DVE recip -> GPSIMD partition broadcast -> DVE mul)
  out[qtile, :] += ZT_pair^T Wo_pair        (partial over local heads)
"""

import numpy as np

import concourse.mybir as mybir
import concourse.tile as tile
from concourse import bacc
from concourse import bass_utils

F32 = mybir.dt.float32
F32R = mybir.dt.float32r

SEQ = 2048
DM = 1024
DH = 64
HLOC = 4          # heads per core
NKC = 8           # dmodel chunks of 128
NQC = 4           # q chunks of 512
QW = 512

_PROGRAMS = {}


def _build(with_bias: bool):
    nc = bacc.Bacc("TRN2", target_bir_lowering=False, debug=False, num_devices=8)

    xTa = nc.dram_tensor("xTa", [DM + 1, SEQ], F32R, kind="ExternalInput").ap()
    wq = nc.dram_tensor("wq", [DM, 256], F32R, kind="ExternalInput").ap()
    wk = nc.dram_tensor("wk", [DM, 256], F32R, kind="ExternalInput").ap()
    wv = nc.dram_tensor("wv", [DM, 256], F32R, kind="ExternalInput").ap()
    wo = nc.dram_tensor("wo", [256, DM], F32R, kind="ExternalInput").ap()
    # [128, 384]: cols 0:128 tri mask (q>=k), 128:256 zeros, 256:384 tri
    tri = nc.dram_tensor("tri", [128, 384], F32, kind="ExternalInput").ap()
    if with_bias:
        bqkv = nc.dram_tensor("bqkv", [1, 768], F32R, kind="ExternalInput").ap()
    out = nc.dram_tensor("out", [SEQ, DM], F32, kind="ExternalOutput").ap()

    with tile.TileContext(nc) as tc:
        with (
            tc.tile_pool(name="px", bufs=1) as px,
            tc.tile_pool(name="pw", bufs=1) as pw,
            tc.tile_pool(name="pqk", bufs=1) as pqk,
            tc.tile_pool(name="pv", bufs=1) as pv,
            tc.tile_pool(name="ppt", bufs=(4 if with_bias else 6)) as ppt,
            tc.tile_pool(name="pzt", bufs=4) as pzt,
            tc.tile_pool(name="prs", bufs=3) as prs,
            tc.tile_pool(name="pout", bufs=(3 if with_bias else 4)) as pout,
            tc.tile_pool(name="psS", bufs=3, space="PSUM") as psS,
            tc.tile_pool(name="psZ", bufs=2, space="PSUM") as psZ,
        ):
            # ---- load inputs (weights interleaved with x so the first
            # projection chain can start as soon as chunk 0 lands) ----
            WQ = pw.tile([128, NKC, 256], F32R, tag="wq")
            WK = pw.tile([128, NKC, 256], F32R, tag="wk")
            WV = pw.tile([128, NKC, 256], F32R, tag="wv")
            # x loaded in column blocks of 512, qc-block-major, so qc=0
            # projections (and attention) start after ~2MB of x instead of 8MB
            X = [[None] * NQC for _ in range(NKC)]
            for kc in range(NKC):
                nc.sync.dma_start(WQ[:, kc, :], wq[kc * 128:(kc + 1) * 128, :])
                nc.sync.dma_start(WK[:, kc, :], wk[kc * 128:(kc + 1) * 128, :])
                xt = px.tile([128, NQC, QW], F32R, tag=f"x{kc}", name=f"xt{kc}")
                for qb in range(NQC):
                    X[kc][qb] = xt[:, qb, :]
                nc.sync.dma_start(
                    X[kc][0], xTa[kc * 128:(kc + 1) * 128, 0:QW])
            for kc in range(NKC):
                nc.sync.dma_start(WV[:, kc, :], wv[kc * 128:(kc + 1) * 128, :])
            for qb in range(1, NQC):
                for kc in range(NKC):
                    nc.sync.dma_start(
                        X[kc][qb],
                        xTa[kc * 128:(kc + 1) * 128, qb * QW:(qb + 1) * QW])
            if with_bias:
                x_ones = px.tile([1, SEQ], F32R, tag="x8")
                nc.sync.dma_start(x_ones[:], xTa[DM:DM + 1, :])

            WO = pw.tile([128, 2, DM], F32R, tag="wo")
            for hp in range(2):
                nc.sync.dma_start(WO[:, hp, :], wo[hp * 128:(hp + 1) * 128, :])
            TRI = pw.tile([128, 384], F32, tag="tri")
            nc.sync.dma_start(TRI[:], tri)
            if with_bias:
                BQKV = pw.tile([1, 768], F32R, tag="bqkv")
                nc.sync.dma_start(BQKV[:], bqkv)

            ones4_f = pw.tile([128, HLOC, 1], F32, tag="ones4")
            nc.any.memset(ones4_f[:], 1.0)

            # ---- projections ----
            # QT/KT: [128 (head-pair), seq], per (hp, qc) tile of [128, 512].
            # Two chains share one 2-bank psum slot so more chains are in
            # flight while x is still streaming in.
            QT = [[None] * NQC for _ in range(2)]
            KT = [[None] * NQC for _ in range(2)]
            chains = []  # (w_sb, bias_off, dst, hp, qc), qc-major
            for qc in range(NQC):
                for w_sb, bias_off, dst in ((WQ, 0, QT), (WK, 256, KT)):
                    for hp in range(2):
                        chains.append((w_sb, bias_off, dst, hp, qc))
            # first two chains run as singles on the psZ banks, which are
            # otherwise idle until attention starts: 8 accumulation chains in
            # flight while x streams in instead of 6
            for j in (0, 1):
                w_sb, bias_off, dst, hp, qc = chains[j]
                pz = psZ.tile([128, QW], F32, tag="zo", name=f"qz{j}")
                for kc in range(NKC):
                    nc.tensor.matmul(
                        pz[:],
                        w_sb[:, kc, hp * 128:(hp + 1) * 128],
                        X[kc][qc],
                        start=(kc == 0),
                        stop=(kc == NKC - 1 and not with_bias),
                    )
                if with_bias:
                    nc.tensor.matmul(
                        pz[:],
                        BQKV[0:1, bias_off + hp * 128:bias_off + (hp + 1) * 128],
                        x_ones[0:1, qc * QW:(qc + 1) * QW],
                        start=False, stop=True,
                    )
                t = pqk.tile([128, QW], F32R,
                             tag=f"{'q' if dst is QT else 'k'}{hp}_{qc}",
                             name=f"tz_{bias_off}_{hp}_{qc}")
                nc.scalar.copy(t[:], pz[:])
                dst[hp][qc] = t
            V1 = [None] * 16

            def emit_qk_pair(j):
                pp = psS.tile([128, 2, QW], F32, tag="s", name=f"qk{j}")
                for kc in range(NKC):
                    for i in (0, 1):
                        w_sb, bias_off, dst, hp, qc = chains[j + i]
                        nc.tensor.matmul(
                            pp[:, i, :],
                            w_sb[:, kc, hp * 128:(hp + 1) * 128],
                            X[kc][qc],
                            start=(kc == 0),
                            stop=(kc == NKC - 1 and not with_bias),
                        )
                for i in (0, 1):
                    w_sb, bias_off, dst, hp, qc = chains[j + i]
                    if with_bias:
                        nc.tensor.matmul(
                            pp[:, i, :],
                            BQKV[0:1, bias_off + hp * 128:
                                 bias_off + (hp + 1) * 128],
                            x_ones[0:1, qc * QW:(qc + 1) * QW],
                            start=False, stop=True,
                        )
                    t = pqk.tile([128, QW], F32R,
                                 tag=f"{'q' if dst is QT else 'k'}{hp}_{qc}",
                                 name=f"t_{bias_off}_{hp}_{qc}")
                    nc.scalar.copy(t[:], pp[:, i, :])
                    dst[hp][qc] = t

            def emit_v_pair(st2):
                # V: [128, 4 heads, 65] per seq-tile (col 64 = ones). Each
                # chain padded to a full psum bank (two accumulation groups
                # must not share a bank).
                pp = psS.tile([128, 2, QW], F32, tag="s", name=f"vq{st2}")
                for kc in range(NKC):
                    for i in (0, 1):
                        st = st2 * 2 + i
                        nc.tensor.matmul(
                            pp[:, i, 0:256],
                            X[kc][st // 4][:, (st % 4) * 128:
                                           (st % 4 + 1) * 128],
                            WV[:, kc, :],
                            start=(kc == 0),
                            stop=(kc == NKC - 1 and not with_bias),
                        )
                for i in (0, 1):
                    st = st2 * 2 + i
                    if with_bias:
                        nc.tensor.matmul(
                            pp[:, i, 0:256],
                            x_ones[0:1, st * 128:(st + 1) * 128],
                            BQKV[0:1, 512:768],
                            start=False, stop=True,
                        )
                    vt = pv.tile([128, HLOC, DH + 1], F32R, tag=f"v{st}",
                                 name=f"vt{st}")
                    nc.vector.tensor_copy(
                        vt[:, :, 0:DH],
                        pp[:, i, 0:256].rearrange("p (h d) -> p h d", h=HLOC),
                    )
                    nc.vector.tensor_copy(vt[:, :, DH:DH + 1], ones4_f[:])
                    V1[st] = vt

            # interleave: per qc-block, its QK pairs then its V pairs, so the
            # qc=0 inputs of attention complete first
            for qc in range(NQC):
                j0 = 4 * qc
                for j in range(max(2, j0), j0 + 4, 2):
                    emit_qk_pair(j)
                emit_v_pair(2 * qc)
                emit_v_pair(2 * qc + 1)

            # ---- attention + output projection, per q-chunk ----
            for qc in range(NQC):
                q0 = qc * QW
                nkt = 4 * qc + 4
                ZT = [None, None]  # per head-pair [128, 512]
                OSB = [None] * 4
                for h in range(HLOC):
                    hp, hh = h // 2, h % 2
                    zps = psZ.tile([128, QW], F32, tag="zo",
                                   name=f"z{qc}_{h}")
                    for ktp in range(nkt // 2):
                        if True:
                            sps = psS.tile([128, 2, QW], F32, tag="s",
                                           name=f"s{qc}_{h}_{ktp}")
                            offs = []
                            for i in (0, 1):
                                kt = 2 * ktp + i
                                # crop to causal region; keep matmul N >= 256
                                # (fp32r below 256 runs at 1/4 rate)
                                off = min(max(0, kt * 128 - q0), 256)
                                offs.append(off)
                                nc.tensor.matmul(
                                    sps[:, i, off:QW],
                                    KT[hp][kt // 4][hh * DH:(hh + 1) * DH,
                                                    (kt % 4) * 128:(kt % 4 + 1) * 128],
                                    QT[hp][qc][hh * DH:(hh + 1) * DH, off:QW],
                                    start=True, stop=True,
                                )
                            pt = ppt.tile([128, 2, QW], F32R, tag="pt",
                                          name=f"pt{qc}_{h}_{ktp}")
                            if offs[0] == 0 and offs[1] == 0:
                                nc.scalar.activation(
                                    pt[:], sps[:],
                                    mybir.ActivationFunctionType.Exp,
                                )
                            else:
                                for i in (0, 1):
                                    nc.scalar.activation(
                                        pt[:, i, offs[i]:QW], sps[:, i, offs[i]:QW],
                                        mybir.ActivationFunctionType.Exp,
                                    )
                        for i in (0, 1):
                            kt = 2 * ktp + i
                            off = offs[i]
                            if kt >= nkt - 4:  # diagonal: mask
                                moff = kt * 128 - q0  # true mask offset
                                if moff == 384:
                                    # cols 256:384 masked to 0, tri on 384:512
                                    nc.vector.tensor_mul(
                                        pt[:, i, 256:512],
                                        pt[:, i, 256:512], TRI[:, 128:384]
                                    )
                                else:
                                    nc.vector.tensor_mul(
                                        pt[:, i, moff:moff + 128],
                                        pt[:, i, moff:moff + 128], TRI[:, 0:128]
                                    )
                            nc.tensor.matmul(
                                zps[0:DH + 1, off:QW],
                                V1[kt][:, h, :],
                                pt[:, i, off:QW],
                                start=(kt == 0),
                                stop=(kt == nkt - 1),
                                skip_group_check=True,
                            )
                    # normalize: ZT[0:64] = zps[0:64] / zps[64]
                    recip = prs.tile([1, QW], F32R, tag="recip",
                                     name=f"rc{qc}_{h}")
                    with nc.allow_low_precision(reason="softmax recip in fp32r"):
                        nc.vector.reciprocal(recip[:], zps[DH:DH + 1, :])
                    rb = prs.tile([DH, QW], F32R, tag="rb", name=f"rb{qc}_{h}")
                    nc.gpsimd.partition_broadcast(rb[:], recip[:])
                    if ZT[hp] is None:
                        ZT[hp] = pzt.tile([128, QW], F32R, tag="zt",
                                          name=f"zt{qc}_{hp}")
                    nc.vector.tensor_mul(
                        ZT[hp][hh * DH:(hh + 1) * DH, :], zps[0:DH, :], rb[:]
                    )

                # out[q0:q0+512, :] = sum_hp ZT[hp].T @ WO[hp]
                for qt in range(4):
                    osb = pout.tile([128, DM], F32, tag="ob",
                                    name=f"ob{qc}_{qt}")
                    for mc in range(2):
                        ops = psZ.tile([128, QW], F32, tag="zo",
                                       name=f"o{qc}_{qt}_{mc}")
                        for hp in range(2):
                            nc.tensor.matmul(
                                ops[:],
                                ZT[hp][:, qt * 128:(qt + 1) * 128],
                                WO[:, hp, mc * QW:(mc + 1) * QW],
                                start=(hp == 0), stop=(hp == 1),
                            )
                        nc.vector.tensor_copy(osb[:, mc * QW:(mc + 1) * QW], ops[:])
                        nc.sync.dma_start(
                            out[q0 + qt * 128:q0 + (qt + 1) * 128,
                                mc * QW:(mc + 1) * QW],
                            osb[:, mc * QW:(mc + 1) * QW],
                        )


    nc.compile()
    return nc


def _get_program(with_bias: bool):
    if with_bias not in _PROGRAMS:
        _PROGRAMS[with_bias] = _build(with_bias)
    return _PROGRAMS[with_bias]


def kernel(normalized_resid_pre, W_Q, W_K, W_V, W_O, b_Q, b_K, b_V, b_O):
    x = np.asarray(normalized_resid_pre, dtype=np.float32)
    W_Q = np.asarray(W_Q, dtype=np.float32)
    W_K = np.asarray(W_K, dtype=np.float32)
    W_V = np.asarray(W_V, dtype=np.float32)
    W_O = np.asarray(W_O, dtype=np.float32)
    b_Q = np.asarray(b_Q, dtype=np.float32)
    b_K = np.asarray(b_K, dtype=np.float32)
    b_V = np.asarray(b_V, dtype=np.float32)
    b_O = np.asarray(b_O, dtype=np.float32)

    batch, seq, dm = x.shape
    with_bias = bool(np.any(b_Q) or np.any(b_K) or np.any(b_V))
    nc = _get_program(with_bias)

    tri1 = np.triu(np.ones((128, 128), dtype=np.float32))
    tri = np.ascontiguousarray(np.concatenate(
        [tri1, np.zeros((128, 128), dtype=np.float32), tri1], axis=1
    ))
    in_maps = []
    for c in range(8):
        b, g = c // 4, c % 4
        hs = slice(4 * g, 4 * g + 4)
        xT = x[b].T  # [1024, 2048]
        xTa = np.concatenate(
            [xT, np.ones((1, seq), dtype=np.float32)], axis=0
        )
        m = {
            "xTa": np.ascontiguousarray(xTa),
            "wq": np.ascontiguousarray(
                np.transpose(W_Q[hs], (1, 0, 2)).reshape(dm, 256)),
            "wk": np.ascontiguousarray(
                np.transpose(W_K[hs], (1, 0, 2)).reshape(dm, 256) * 0.125),
            "wv": np.ascontiguousarray(
                np.transpose(W_V[hs], (1, 0, 2)).reshape(dm, 256)),
            "wo": np.ascontiguousarray(W_O[hs].reshape(256, dm)),
            "tri": tri,
        }
        if with_bias:
            m["bqkv"] = np.ascontiguousarray(np.concatenate(
                [b_Q[hs].reshape(256), b_K[hs].reshape(256) * 0.125,
                 b_V[hs].reshape(256)]
            )[None, :].astype(np.float32))
        in_maps.append(m)

    res = bass_utils.run_bass_kernel_spmd(nc, in_maps, core_ids=list(range(8)))
    parts = [res.results[c]["out"] for c in range(8)]
    full = np.stack(
        [parts[0] + parts[1] + parts[2] + parts[3],
         parts[4] + parts[5] + parts[6] + parts[7]]
    )
    full += b_O
    return full.astype(np.float32)



# revision 8
# speedup vs baseline: 1.1020x; 1.1020x over previous
"""Multi-head causal attention (B=2, S=2048, D=1024, H=16, Dh=64) on 8 TRN2 cores.

Sharding: tensor-parallel over 4 head-groups x data-parallel over 2 batches.
Core c handles batch c//4, heads [4*(c%4), 4*(c%4)+4); host sums the 4 partial
output projections per batch and adds b_O.

Per-core program (fast path, zero qkv biases):
  Q/K projections: fp8 DoubleRow matmuls (x8, W8 at scale 8) -> psum f32 ->
    fp8 QT8/KT8 in [4h x 32dh, 2 half, seq] layout (host pre-permutes W cols).
  V projection: bf16 (x bf16, Wv bf16*8) -> V [seq, h, 65] bf16 (col 64 = 1).
  Scores: DoubleRow fp8, contraction dh=2x32, S'' = 512*S_true in psum.
    Causal mask added pre-exp by a DR matmul (32*I @ -240 tri) on diag tiles.
  Softmax exp: split ACT (activation Exp, scale 1/512) and
    DVE-stage + GPSIMD pow(2, s*log2e/512) to balance engines.
  PV: flipped orientation z[q, h, 65] += PT[k, q-blk]^T V[k, 65], bf16.
  Normalize: DVE reciprocal of col 64 + broadcast multiply -> ZN bf16.
  ZN^T via PE transpose (identity matmul) -> ZT bf16 -> out proj bf16 ->
    psum f32 -> bf16 staging (ACT copy) -> DMA out (bf16, x64 scale).
"""

import numpy as np
import ml_dtypes

import concourse.mybir as mybir
import concourse.tile as tile
from concourse import bacc
from concourse import bass_utils

F32 = mybir.dt.float32
F32R = mybir.dt.float32r
BF16 = mybir.dt.bfloat16
F8 = mybir.dt.float8e4
E4NP = ml_dtypes.float8_e4m3
BFNP = ml_dtypes.bfloat16

SEQ = 2048
DM = 1024
DH = 64
HLOC = 4
NQC = 4
QW = 512
LOG2E = float(np.log2(np.e))

_PROGRAMS = {}


def _build_fast():
    nc = bacc.Bacc("TRN2", target_bir_lowering=False, debug=False, num_devices=8)

    d_x8 = nc.dram_tensor("x8p", [128, 16384], F8, kind="ExternalInput").ap()
    d_xbf = nc.dram_tensor("xbf", [128, 16384], BF16, kind="ExternalInput").ap()
    d_wqk = nc.dram_tensor("wqk8", [128, 4096], F8, kind="ExternalInput").ap()
    d_wv = nc.dram_tensor("wvbf", [128, 2048], BF16, kind="ExternalInput").ap()
    d_wo = nc.dram_tensor("wobf", [128, 2048], BF16, kind="ExternalInput").ap()
    d_msk = nc.dram_tensor("msk", [64, 512], F8, kind="ExternalInput").ap()
    d_id = nc.dram_tensor("identb", [128, 128], BF16, kind="ExternalInput").ap()
    d_out = nc.dram_tensor("out", [SEQ, DM], BF16, kind="ExternalOutput").ap()

    x8v = d_x8.rearrange("p (a t s) -> p a t s", a=4, t=2, s=SEQ)
    xbv = d_xbf.rearrange("p (k s) -> p k s", k=8, s=SEQ)

    with tile.TileContext(nc) as tc:
        with (
            tc.tile_pool(name="pw", bufs=1) as pw,
            tc.tile_pool(name="px", bufs=1) as px,
            tc.tile_pool(name="pqk", bufs=1) as pqk,
            tc.tile_pool(name="pv", bufs=1) as pv,
            tc.tile_pool(name="ppt", bufs=6) as ppt,
            tc.tile_pool(name="pse", bufs=3) as pse,
            tc.tile_pool(name="pzn", bufs=6) as pzn,
            tc.tile_pool(name="pzt", bufs=2) as pzt,
            tc.tile_pool(name="pos", bufs=3) as pos,
            tc.tile_pool(name="psS", bufs=2, space="PSUM") as psS,
            tc.tile_pool(name="psZ", bufs=4, space="PSUM") as psZ,
        ):
            # ---- constants + PE warmup (runs while DMAs land) ----
            WU = pw.tile([128, 2, 128], F8, tag="wu")
            nc.vector.memset(WU[:], 0.0)
            C2 = pw.tile([128, 1], F32, tag="c2")
            nc.vector.memset(C2[:], 2.0)

            # ---- input DMAs (SP HWDGE) ----
            WQK = pw.tile([128, 2, 4, 2, 2, 128], F8, tag="wqk")
            nc.sync.dma_start(WQK[:], d_wqk)
            X8 = px.tile([128, 4, 2, SEQ], F8, tag="x8")
            for qc in range(NQC):
                nc.sync.dma_start(X8[:, :, :, qc * QW:(qc + 1) * QW],
                                  x8v[:, :, :, qc * QW:(qc + 1) * QW])
            MSK = pw.tile([64, 2, 2, 128], F8, tag="msk")
            nc.sync.dma_start(MSK[:], d_msk)
            IDB = pw.tile([128, 128], BF16, tag="idb")
            nc.sync.dma_start(IDB[:], d_id)
            WV = pw.tile([128, 8, 256], BF16, tag="wv")
            nc.sync.dma_start(WV[:], d_wv)
            XB = px.tile([128, 8, SEQ], BF16, tag="xb")
            for qc in range(NQC):
                nc.sync.dma_start(XB[:, :, qc * QW:(qc + 1) * QW],
                                  xbv[:, :, qc * QW:(qc + 1) * QW])
            WO = pw.tile([128, 2, DM], BF16, tag="wo")
            nc.sync.dma_start(WO[:], d_wo)

            # PE warmup: ramp the pstate clock before real chains arrive
            wps = psZ.tile([128, QW], F32, tag="z", name="warm")
            for i in range(64):
                nc.tensor.matmul(
                    wps[:, 0:128], WU[:], WU[:],
                    perf_mode=mybir.MatmulPerfMode.DoubleRow,
                    start=True, stop=True, skip_group_check=True)

            # ---- Q/K projections (fp8 DoubleRow) ----
            # SBUF APs can only start at partition 0/32/64, so heads split
            # across two tiles: [h//2] holds heads (2hp, 2hp+1) at bases 0/32.
            QK8 = [[pqk.tile([64, 2, SEQ], F8, tag=f"qk8_{w}_{hp}",
                             name=f"qk8_{w}_{hp}")
                    for hp in range(2)] for w in range(2)]
            for qc in range(NQC):
                for w in range(2):
                    for j in range(2):
                        for hp in range(2):
                            pp = psZ.tile([64, QW], F32, tag="z",
                                          name=f"qk{qc}_{w}_{j}_{hp}")
                            for a in range(4):
                                nc.tensor.matmul(
                                    pp[:],
                                    WQK[:, w, a, :, j,
                                        64 * hp:64 * hp + 64],
                                    X8[:, a, :, qc * QW:(qc + 1) * QW],
                                    perf_mode=mybir.MatmulPerfMode.DoubleRow,
                                    start=(a == 0), stop=(a == 3))
                            nc.vector.tensor_copy(
                                QK8[w][hp][:, j, qc * QW:(qc + 1) * QW],
                                pp[:])

            # ---- V projection (bf16), interleaved with attention below ----
            V = pv.tile([128, 16, HLOC, DH + 1], BF16, tag="v")
            nc.gpsimd.memset(V[:, :, :, DH], 1.0)

            def emit_v(st):
                pp = psZ.tile([128, 256], F32, tag="z", name=f"v{st}")
                for kc in range(8):
                    nc.tensor.matmul(
                        pp[:],
                        XB[:, kc, st * 128:(st + 1) * 128],
                        WV[:, kc, :],
                        start=(kc == 0), stop=(kc == 7))
                nc.vector.tensor_copy(
                    V[:, st, :, 0:DH],
                    pp[:].rearrange("p (h d) -> p h d", h=HLOC))

            # ---- attention per q-chunk ----
            exp_ctr = [0]

            def emit_attn(qc):
                q0 = qc * QW
                nkt = 4 * qc + 4
                zq = [psZ.tile([128, HLOC, DH + 1], F32, tag="z",
                               name=f"z{qc}_{qt}") for qt in range(4)]
                for h in range(HLOC):
                    for ktp in range(nkt // 2):
                        sps = psS.tile([128, 2, QW], F32, tag="s",
                                       name=f"s{qc}_{h}_{ktp}")
                        hp, hb = h // 2, 32 * (h % 2)
                        offs = []
                        for i in (0, 1):
                            kt = 2 * ktp + i
                            off = max(0, kt * 128 - q0)
                            offs.append(off)
                            nc.tensor.matmul(
                                sps[:, i, off:QW],
                                QK8[1][hp][hb:hb + 32, :,
                                           kt * 128:(kt + 1) * 128],
                                QK8[0][hp][hb:hb + 32, :, q0 + off:q0 + QW],
                                perf_mode=mybir.MatmulPerfMode.DoubleRow,
                                start=True, stop=(kt < 4 * qc),
                                skip_group_check=True)
                            if kt >= 4 * qc:  # diagonal: add -7680 mask
                                nc.tensor.matmul(
                                    sps[:, i, off:off + 128],
                                    MSK[:, :, 0, :], MSK[:, :, 1, :],
                                    perf_mode=mybir.MatmulPerfMode.DoubleRow,
                                    start=False, stop=True,
                                    skip_group_check=True)
                        pt = ppt.tile([128, 2, QW], BF16, tag="pt",
                                      name=f"pt{qc}_{h}_{ktp}")
                        poff = offs[0]
                        exp_ctr[0] += 1
                        if exp_ctr[0] % 3 == 0:
                            # offload: DVE stages scaled scores, GPSIMD pows
                            se = pse.tile([128, 2, QW], F32, tag="se",
                                          name=f"se{qc}_{h}_{ktp}")
                            nc.vector.tensor_scalar_mul(
                                se[:, :, poff:QW], sps[:, :, poff:QW],
                                LOG2E / 512.0)
                            nc.gpsimd.tensor_tensor(
                                pt[:, :, poff:QW],
                                C2[:].broadcast_to([128, 2, QW - poff]),
                                se[:, :, poff:QW],
                                op=mybir.AluOpType.pow)
                        else:
                            nc.scalar.activation(
                                pt[:, :, poff:QW], sps[:, :, poff:QW],
                                mybir.ActivationFunctionType.Exp,
                                scale=1.0 / 512.0)
                        # PV (flipped): z[q, h, :] += PT[:, qblk]^T V[kt]
                        for i in (0, 1):
                            kt = 2 * ktp + i
                            for qt in range(4):
                                if 4 * qc + qt < kt:
                                    continue
                                nc.tensor.matmul(
                                    zq[qt][:, h, :],
                                    pt[:, i, qt * 128:(qt + 1) * 128],
                                    V[:, kt, h, :],
                                    start=(kt == 0),
                                    stop=(kt == 4 * qc + qt),
                                    skip_group_check=True)

                # normalize + transpose + output projection
                ZT = pzt.tile([128, 2, QW], BF16, tag="zt", name=f"zt{qc}")
                for qt in range(4):
                    rc = pzn.tile([128, HLOC], F32, tag="rc",
                                  name=f"rc{qc}_{qt}")
                    nc.vector.reciprocal(rc[:], zq[qt][:, :, DH])
                    zn = pzn.tile([128, HLOC, DH], BF16, tag="zn",
                                  name=f"zn{qc}_{qt}")
                    nc.vector.tensor_tensor(
                        zn[:], zq[qt][:, :, 0:DH],
                        rc[:].unsqueeze(2).broadcast_to([128, HLOC, DH]),
                        op=mybir.AluOpType.mult)
                    tr = psZ.tile([128, 2, 128], BF16, tag="z",
                                  name=f"tr{qc}_{qt}")
                    for hp in range(2):
                        nc.tensor.transpose(
                            tr[:, hp, :],
                            zn[:, 2 * hp:2 * hp + 2, :], IDB[:])
                    nc.vector.tensor_copy(
                        ZT[:, :, qt * 128:(qt + 1) * 128], tr[:])
                for qt in range(4):
                    osb = pos.tile([128, DM], BF16, tag="os",
                                   name=f"os{qc}_{qt}")
                    for mc in range(2):
                        po = psZ.tile([128, QW], F32, tag="z",
                                      name=f"o{qc}_{qt}_{mc}")
                        for hp in range(2):
                            nc.tensor.matmul(
                                po[:],
                                ZT[:, hp, qt * 128:(qt + 1) * 128],
                                WO[:, hp, mc * QW:(mc + 1) * QW],
                                start=(hp == 0), stop=(hp == 1))
                        nc.scalar.copy(osb[:, mc * QW:(mc + 1) * QW], po[:])
                    nc.sync.dma_start(
                        d_out[q0 + qt * 128:q0 + (qt + 1) * 128, :], osb[:])

            # schedule: V tiles just ahead of the attention chunk needing them
            for st in range(8):
                emit_v(st)
            emit_attn(0)
            for st in range(8, 12):
                emit_v(st)
            emit_attn(1)
            for st in range(12, 16):
                emit_v(st)
            emit_attn(2)
            emit_attn(3)

    nc.compile()
    return nc


def _build_bias_prog():
    """Fallback program for nonzero q/k/v biases (fp32r, slower, from the
    original implementation)."""
    nc = bacc.Bacc("TRN2", target_bir_lowering=False, debug=False, num_devices=8)
    NKC = 8
    HL = 4

    xTa = nc.dram_tensor("xTa", [DM + 1, SEQ], F32R, kind="ExternalInput").ap()
    wq = nc.dram_tensor("wq", [DM, 256], F32R, kind="ExternalInput").ap()
    wk = nc.dram_tensor("wk", [DM, 256], F32R, kind="ExternalInput").ap()
    wv = nc.dram_tensor("wv", [DM, 256], F32R, kind="ExternalInput").ap()
    wo = nc.dram_tensor("wo", [256, DM], F32R, kind="ExternalInput").ap()
    tri = nc.dram_tensor("tri", [128, 384], F32, kind="ExternalInput").ap()
    bqkv = nc.dram_tensor("bqkv", [1, 768], F32R, kind="ExternalInput").ap()
    out = nc.dram_tensor("out", [SEQ, DM], F32, kind="ExternalOutput").ap()

    with tile.TileContext(nc) as tc:
        with (
            tc.tile_pool(name="px", bufs=1) as px,
            tc.tile_pool(name="pw", bufs=1) as pw,
            tc.tile_pool(name="pqk", bufs=1) as pqk,
            tc.tile_pool(name="pv", bufs=1) as pv,
            tc.tile_pool(name="ppt", bufs=4) as ppt,
            tc.tile_pool(name="pzt", bufs=4) as pzt,
            tc.tile_pool(name="prs", bufs=3) as prs,
            tc.tile_pool(name="pout", bufs=3) as pout,
            tc.tile_pool(name="psS", bufs=3, space="PSUM") as psS,
            tc.tile_pool(name="psZ", bufs=2, space="PSUM") as psZ,
        ):
            WQ = pw.tile([128, NKC, 256], F32R, tag="wq")
            WK = pw.tile([128, NKC, 256], F32R, tag="wk")
            WV = pw.tile([128, NKC, 256], F32R, tag="wv")
            X = [[None] * NQC for _ in range(NKC)]
            for kc in range(NKC):
                nc.sync.dma_start(WQ[:, kc, :], wq[kc * 128:(kc + 1) * 128, :])
                nc.sync.dma_start(WK[:, kc, :], wk[kc * 128:(kc + 1) * 128, :])
                xt = px.tile([128, NQC, QW], F32R, tag=f"x{kc}", name=f"xt{kc}")
                for qb in range(NQC):
                    X[kc][qb] = xt[:, qb, :]
                nc.sync.dma_start(X[kc][0], xTa[kc * 128:(kc + 1) * 128, 0:QW])
            for kc in range(NKC):
                nc.sync.dma_start(WV[:, kc, :], wv[kc * 128:(kc + 1) * 128, :])
            for qb in range(1, NQC):
                for kc in range(NKC):
                    nc.sync.dma_start(
                        X[kc][qb],
                        xTa[kc * 128:(kc + 1) * 128, qb * QW:(qb + 1) * QW])
            x_ones = px.tile([1, SEQ], F32R, tag="x8")
            nc.sync.dma_start(x_ones[:], xTa[DM:DM + 1, :])

            WO = pw.tile([128, 2, DM], F32R, tag="wo")
            for hp in range(2):
                nc.sync.dma_start(WO[:, hp, :], wo[hp * 128:(hp + 1) * 128, :])
            TRI = pw.tile([128, 384], F32, tag="tri")
            nc.sync.dma_start(TRI[:], tri)
            BQKV = pw.tile([1, 768], F32R, tag="bqkv")
            nc.sync.dma_start(BQKV[:], bqkv)

            ones4_f = pw.tile([128, HL, 1], F32, tag="ones4")
            nc.any.memset(ones4_f[:], 1.0)

            QT = [[None] * NQC for _ in range(2)]
            KT = [[None] * NQC for _ in range(2)]
            chains = []
            for qc in range(NQC):
                for w_sb, bias_off, dst in ((WQ, 0, QT), (WK, 256, KT)):
                    for hp in range(2):
                        chains.append((w_sb, bias_off, dst, hp, qc))
            for j in (0, 1):
                w_sb, bias_off, dst, hp, qc = chains[j]
                pz = psZ.tile([128, QW], F32, tag="zo", name=f"qz{j}")
                for kc in range(NKC):
                    nc.tensor.matmul(
                        pz[:], w_sb[:, kc, hp * 128:(hp + 1) * 128],
                        X[kc][qc], start=(kc == 0), stop=False)
                nc.tensor.matmul(
                    pz[:],
                    BQKV[0:1, bias_off + hp * 128:bias_off + (hp + 1) * 128],
                    x_ones[0:1, qc * QW:(qc + 1) * QW],
                    start=False, stop=True)
                t = pqk.tile([128, QW], F32R,
                             tag=f"{'q' if dst is QT else 'k'}{hp}_{qc}",
                             name=f"tz_{bias_off}_{hp}_{qc}")
                nc.scalar.copy(t[:], pz[:])
                dst[hp][qc] = t
            V1 = [None] * 16

            def emit_qk_pair(j):
                pp = psS.tile([128, 2, QW], F32, tag="s", name=f"qk{j}")
                for kc in range(NKC):
                    for i in (0, 1):
                        w_sb, bias_off, dst, hp, qc = chains[j + i]
                        nc.tensor.matmul(
                            pp[:, i, :],
                            w_sb[:, kc, hp * 128:(hp + 1) * 128],
                            X[kc][qc], start=(kc == 0), stop=False)
                for i in (0, 1):
                    w_sb, bias_off, dst, hp, qc = chains[j + i]
                    nc.tensor.matmul(
                        pp[:, i, :],
                        BQKV[0:1, bias_off + hp * 128:bias_off + (hp + 1) * 128],
                        x_ones[0:1, qc * QW:(qc + 1) * QW],
                        start=False, stop=True)
                    t = pqk.tile([128, QW], F32R,
                                 tag=f"{'q' if dst is QT else 'k'}{hp}_{qc}",
                                 name=f"t_{bias_off}_{hp}_{qc}")
                    nc.scalar.copy(t[:], pp[:, i, :])
                    dst[hp][qc] = t

            def emit_v_pair(st2):
                pp = psS.tile([128, 2, QW], F32, tag="s", name=f"vq{st2}")
                for kc in range(NKC):
                    for i in (0, 1):
                        st = st2 * 2 + i
                        nc.tensor.matmul(
                            pp[:, i, 0:256],
                            X[kc][st // 4][:, (st % 4) * 128:(st % 4 + 1) * 128],
                            WV[:, kc, :], start=(kc == 0), stop=False)
                for i in (0, 1):
                    st = st2 * 2 + i
                    nc.tensor.matmul(
                        pp[:, i, 0:256],
                        x_ones[0:1, st * 128:(st + 1) * 128],
                        BQKV[0:1, 512:768], start=False, stop=True)
                    vt = pv.tile([128, HL, DH + 1], F32R, tag=f"v{st}",
                                 name=f"vt{st}")
                    nc.vector.tensor_copy(
                        vt[:, :, 0:DH],
                        pp[:, i, 0:256].rearrange("p (h d) -> p h d", h=HL))
                    nc.vector.tensor_copy(vt[:, :, DH:DH + 1], ones4_f[:])
                    V1[st] = vt

            for qc in range(NQC):
                j0 = 4 * qc
                for j in range(max(2, j0), j0 + 4, 2):
                    emit_qk_pair(j)
                emit_v_pair(2 * qc)
                emit_v_pair(2 * qc + 1)

            for qc in range(NQC):
                q0 = qc * QW
                nkt = 4 * qc + 4
                ZT = [None, None]
                for h in range(HL):
                    hp, hh = h // 2, h % 2
                    zps = psZ.tile([128, QW], F32, tag="zo", name=f"z{qc}_{h}")
                    for ktp in range(nkt // 2):
                        sps = psS.tile([128, 2, QW], F32, tag="s",
                                       name=f"s{qc}_{h}_{ktp}")
                        offs = []
                        for i in (0, 1):
                            kt = 2 * ktp + i
                            off = min(max(0, kt * 128 - q0), 256)
                            offs.append(off)
                            nc.tensor.matmul(
                                sps[:, i, off:QW],
                                KT[hp][kt // 4][hh * DH:(hh + 1) * DH,
                                                (kt % 4) * 128:(kt % 4 + 1) * 128],
                                QT[hp][qc][hh * DH:(hh + 1) * DH, off:QW],
                                start=True, stop=True)
                        pt = ppt.tile([128, 2, QW], F32R, tag="pt",
                                      name=f"pt{qc}_{h}_{ktp}")
                        if offs[0] == 0 and offs[1] == 0:
                            nc.scalar.activation(
                                pt[:], sps[:], mybir.ActivationFunctionType.Exp)
                        else:
                            for i in (0, 1):
                                nc.scalar.activation(
                                    pt[:, i, offs[i]:QW], sps[:, i, offs[i]:QW],
                                    mybir.ActivationFunctionType.Exp)
                        for i in (0, 1):
                            kt = 2 * ktp + i
                            off = offs[i]
                            if kt >= nkt - 4:
                                moff = kt * 128 - q0
                                if moff == 384:
                                    nc.vector.tensor_mul(
                                        pt[:, i, 256:512],
                                        pt[:, i, 256:512], TRI[:, 128:384])
                                else:
                                    nc.vector.tensor_mul(
                                        pt[:, i, moff:moff + 128],
                                        pt[:, i, moff:moff + 128], TRI[:, 0:128])
                            nc.tensor.matmul(
                                zps[0:DH + 1, off:QW],
                                V1[kt][:, h, :], pt[:, i, off:QW],
                                start=(kt == 0), stop=(kt == nkt - 1),
                                skip_group_check=True)
                    recip = prs.tile([1, QW], F32R, tag="recip",
                                     name=f"rc{qc}_{h}")
                    with nc.allow_low_precision(reason="softmax recip in fp32r"):
                        nc.vector.reciprocal(recip[:], zps[DH:DH + 1, :])
                    rb = prs.tile([DH, QW], F32R, tag="rb", name=f"rb{qc}_{h}")
                    nc.gpsimd.partition_broadcast(rb[:], recip[:])
                    if ZT[hp] is None:
                        ZT[hp] = pzt.tile([128, QW], F32R, tag="zt",
                                          name=f"zt{qc}_{hp}")
                    nc.vector.tensor_mul(
                        ZT[hp][hh * DH:(hh + 1) * DH, :], zps[0:DH, :], rb[:])

                for qt in range(4):
                    osb = pout.tile([128, DM], F32, tag="ob",
                                    name=f"ob{qc}_{qt}")
                    for mc in range(2):
                        ops = psZ.tile([128, QW], F32, tag="zo",
                                       name=f"o{qc}_{qt}_{mc}")
                        for hp in range(2):
                            nc.tensor.matmul(
                                ops[:], ZT[hp][:, qt * 128:(qt + 1) * 128],
                                WO[:, hp, mc * QW:(mc + 1) * QW],
                                start=(hp == 0), stop=(hp == 1))
                        nc.vector.tensor_copy(osb[:, mc * QW:(mc + 1) * QW], ops[:])
                        nc.sync.dma_start(
                            out[q0 + qt * 128:q0 + (qt + 1) * 128,
                                mc * QW:(mc + 1) * QW],
                            osb[:, mc * QW:(mc + 1) * QW])

    nc.compile()
    return nc


def _kernel_bias(x, W_Q, W_K, W_V, W_O, b_Q, b_K, b_V, b_O):
    nc = _get_program(True)
    seq, dm = SEQ, DM
    tri1 = np.triu(np.ones((128, 128), dtype=np.float32))
    tri = np.ascontiguousarray(np.concatenate(
        [tri1, np.zeros((128, 128), dtype=np.float32), tri1], axis=1))
    in_maps = []
    for c in range(8):
        b, g = c // 4, c % 4
        hs = slice(4 * g, 4 * g + 4)
        xT = x[b].T
        xTa = np.concatenate([xT, np.ones((1, seq), dtype=np.float32)], axis=0)
        m = {
            "xTa": np.ascontiguousarray(xTa),
            "wq": np.ascontiguousarray(
                np.transpose(W_Q[hs], (1, 0, 2)).reshape(dm, 256)),
            "wk": np.ascontiguousarray(
                np.transpose(W_K[hs], (1, 0, 2)).reshape(dm, 256) * 0.125),
            "wv": np.ascontiguousarray(
                np.transpose(W_V[hs], (1, 0, 2)).reshape(dm, 256)),
            "wo": np.ascontiguousarray(W_O[hs].reshape(256, dm)),
            "tri": tri,
            "bqkv": np.ascontiguousarray(np.concatenate(
                [b_Q[hs].reshape(256), b_K[hs].reshape(256) * 0.125,
                 b_V[hs].reshape(256)])[None, :].astype(np.float32)),
        }
        in_maps.append(m)
    res = bass_utils.run_bass_kernel_spmd(nc, in_maps, core_ids=list(range(8)))
    parts = [res.results[c]["out"] for c in range(8)]
    full = np.stack(
        [parts[0] + parts[1] + parts[2] + parts[3],
         parts[4] + parts[5] + parts[6] + parts[7]])
    full += b_O
    return full.astype(np.float32)


def _get_program(with_bias: bool):
    key = "bias" if with_bias else "fast"
    if key not in _PROGRAMS:
        _PROGRAMS[key] = _build_bias_prog() if with_bias else _build_fast()
    return _PROGRAMS[key]


def _prep_core(x, W_Q, W_K, W_V, W_O, hs, b):
    xT = np.ascontiguousarray(x[b].T)  # [1024, 2048] f32
    x8 = xT.astype(E4NP)
    x8p = np.ascontiguousarray(
        x8.reshape(4, 2, 128, SEQ).transpose(2, 0, 1, 3)).reshape(128, 16384)
    xbf = np.ascontiguousarray(
        xT.astype(BFNP).reshape(8, 128, SEQ).transpose(1, 0, 2)
    ).reshape(128, 16384)

    def qk_pack(W):
        w8 = (8.0 * W[hs]).astype(E4NP)  # [4, 1024, 64]
        t = w8.reshape(4, 4, 2, 128, 2, 32).transpose(3, 1, 2, 4, 0, 5)
        return np.ascontiguousarray(t).reshape(128, 2048)

    wqk8 = np.ascontiguousarray(
        np.stack([qk_pack(W_Q), qk_pack(W_K)], axis=1)).reshape(128, 4096)
    wvbf = np.ascontiguousarray(
        (8.0 * W_V[hs]).astype(BFNP).reshape(4, 8, 128, 64)
        .transpose(2, 1, 0, 3)).reshape(128, 2048)
    wobf = np.ascontiguousarray(
        (8.0 * W_O[hs]).astype(BFNP).reshape(2, 2, 64, DM)
        .transpose(1, 2, 0, 3)).reshape(128, 2048)
    return {"x8p": x8p, "xbf": xbf, "wqk8": wqk8, "wvbf": wvbf, "wobf": wobf}


def kernel(normalized_resid_pre, W_Q, W_K, W_V, W_O, b_Q, b_K, b_V, b_O):
    x = np.asarray(normalized_resid_pre, dtype=np.float32)
    W_Q = np.asarray(W_Q, dtype=np.float32)
    W_K = np.asarray(W_K, dtype=np.float32)
    W_V = np.asarray(W_V, dtype=np.float32)
    W_O = np.asarray(W_O, dtype=np.float32)
    b_Q = np.asarray(b_Q, dtype=np.float32)
    b_K = np.asarray(b_K, dtype=np.float32)
    b_V = np.asarray(b_V, dtype=np.float32)
    b_O = np.asarray(b_O, dtype=np.float32)

    with_bias = bool(np.any(b_Q) or np.any(b_K) or np.any(b_V))
    if with_bias:
        return _kernel_bias(x, W_Q, W_K, W_V, W_O, b_Q, b_K, b_V, b_O)

    nc = _get_program(False)

    # static mask operands: 32*I (DR-split) and -240 lower-exclusive tri
    I32 = np.zeros((64, 2, 128), dtype=np.float32)
    for t in range(2):
        for p in range(64):
            I32[p, t, 64 * t + p] = 32.0
    M8 = np.zeros((64, 2, 128), dtype=np.float32)
    for t in range(2):
        for p in range(64):
            M8[p, t, :64 * t + p] = -240.0
    msk = np.ascontiguousarray(
        np.stack([I32, M8], axis=2)).reshape(64, 512).astype(E4NP)
    identb = np.eye(128).astype(BFNP)

    in_maps = []
    for c in range(8):
        b, g = c // 4, c % 4
        m = _prep_core(x, W_Q, W_K, W_V, W_O, slice(4 * g, 4 * g + 4), b)
        m["msk"] = msk
        m["identb"] = identb
        in_maps.append(m)

    res = bass_utils.run_bass_kernel_spmd(nc, in_maps, core_ids=list(range(8)))
    parts = [res.results[c]["out"].astype(np.float32) for c in range(8)]
    full = np.stack(
        [parts[0] + parts[1] + parts[2] + parts[3],
         parts[4] + parts[5] + parts[6] + parts[7]]
    ) * (1.0 / 64.0)
    full += b_O
    return full.astype(np.float32)
